# revision 1
# baseline (speedup 1.0000x reference)
"""Trainium2 Bass kernel for nn_GaussianBlur: depthwise 2D conv, 71x71 kernel,
x [16,3,512,512] fp32.

Strategy:
  - The 71x71 kernel is (numerically) low-rank; decompose it via SVD into r
    separable rank-1 components (r=1 for a Gaussian or all-ones kernel).
  - Each component's 2D conv = 1D conv along H then 1D conv along W. Each 1D
    conv (with zero padding baked in) is a banded 512x512 Toeplitz matmul:
        Y = sum_i A_i @ X @ B_i
    computed on TensorE as two chained matmuls with no transposes:
        Tt_i = X^T @ A_i^T   (lhsT = X,    rhs = A_i^T)
        Y   += Tt_i^T @ B_i  (lhsT = Tt_i, rhs = B_i)
  - float32r matmul mode (fp32 storage, 1 cycle/row at N>=512).
  - Data parallel: 48 (n,c) slices sharded 6-per-core across 8 NeuronCores.
"""

import sys

sys.path.insert(0, "/opt/trn_rl_repo")

from contextlib import ExitStack

import numpy as np

import concourse.bass as bass
import concourse.tile as tile
from concourse import bacc, mybir
from concourse.bass import ts
from concourse.bass_utils import run_bass_kernel_spmd

N_CORES = 8
H = W = 512
PT = 128          # partition tile
NT = H // PT      # 4 tiles per 512 dim
SLICES_PER_CORE = 6  # 16*3 / 8
PAD = 35
KS = 71

_kernel_cache = {}


def _build_bass(r: int):
    """Build + compile the per-core Bass module for r separable components."""
    f32 = mybir.dt.float32
    f32r = mybir.dt.float32r

    nc = bacc.Bacc(name="gaussblur")
    x_d = nc.dram_tensor("x", [SLICES_PER_CORE, H, W], f32r, kind="ExternalInput")
    at_d = nc.dram_tensor("at", [r, H, H], f32r, kind="ExternalInput")
    b_d = nc.dram_tensor("b", [r, W, W], f32r, kind="ExternalInput")
    y_d = nc.dram_tensor("y", [SLICES_PER_CORE, H, W], f32, kind="ExternalOutput")

    with tile.TileContext(nc) as tc, ExitStack() as ctx:
        const_pool = ctx.enter_context(tc.tile_pool(name="const", bufs=1))
        x_pool = ctx.enter_context(tc.tile_pool(name="xp", bufs=3))
        tt_pool = ctx.enter_context(tc.tile_pool(name="ttp", bufs=2))
        y_pool = ctx.enter_context(tc.tile_pool(name="yp", bufs=2))
        ps1 = ctx.enter_context(tc.tile_pool(name="ps1", bufs=4, space="PSUM"))
        ps2 = ctx.enter_context(tc.tile_pool(name="ps2", bufs=4, space="PSUM"))

        # Constants: band matrices, SBUF layout [p, i, ktile, n] with
        # row k = ktile*128 + p. HWDGE queues only (sync + scalar) — SWDGE
        # (gpsimd) adds a ~3us drain at kernel exit. Slice-0 x chunks and the
        # at chunks interleave across both queues so the first matmuls (which
        # need x/at chunks 0..2 within ~1us) never stall.
        at_t = const_pool.tile([PT, r, NT, H], f32r)
        b_t = const_pool.tile([PT, r, NT, W], f32r)

        # Banded accumulation: the Toeplitz band (half-width 35 < 128) means a
        # 256-col output region only needs 3 of the 4 k-tiles. One start=True
        # per PSUM bank clears has_written for the whole bank; later matmuls
        # (start=False) overwrite-where-unset / accumulate-where-set, so
        # per-region groups inside one bank are safe.
        RG = 256
        REGIONS = [(0, (0, 1, 2)), (RG, (1, 2, 3))]

        def banded_mms(out_ps, lhsT_of_tk, rhs_of_tk_cols):
            n_mm = sum(len(tks) for _, tks in REGIONS)
            cnt = 0
            for c0, tks in REGIONS:
                for tk in tks:
                    nc.tensor.matmul(
                        out_ps[:, c0 : c0 + RG],
                        lhsT_of_tk(tk),
                        rhs_of_tk_cols(tk, c0),
                        start=(cnt == 0),
                        stop=(cnt == n_mm - 1),
                    )
                    cnt += 1

        for s in range(SLICES_PER_CORE):
            # x chunked by row-tile: contiguous 256KB DMAs; compute on chunk
            # tk can start as soon as that chunk lands.
            x_t = x_pool.tile([PT, NT, W], f32r)
            if s == 0:
                # Interleave x/at chunk loads across both HWDGE queues in
                # consumption order, then the b chunks (needed ~8us later).
                for tk in range(NT):
                    qx, qa = (nc.sync, nc.scalar) if tk % 2 == 0 else (nc.scalar, nc.sync)
                    qx.dma_start(x_t[:, tk, :], x_d.ap()[s, ts(tk, PT), :])
                    for i in range(r):
                        qa.dma_start(
                            at_t[:, i, tk, :], at_d.ap()[i, ts(tk, PT), :]
                        )
                for tk in range(NT):
                    q = nc.sync if tk % 2 == 0 else nc.scalar
                    for i in range(r):
                        q.dma_start(b_t[:, i, tk, :], b_d.ap()[i, ts(tk, PT), :])
            else:
                for tk in range(NT):
                    nc.sync.dma_start(x_t[:, tk, :], x_d.ap()[s, ts(tk, PT), :])

            # Pass 1: Tt_i = X^T @ A_i^T  -> [w, h'] layout. tm-major: each
            # out tile's copy overlaps the next tile's matmuls.
            tt_t = tt_pool.tile([PT, r, NT, H], f32r)
            for i in range(r):
                for tm in range(NT):
                    o1 = ps1.tile([PT, H], f32, name="o1", tag="o1")
                    banded_mms(
                        o1,
                        lambda tk: x_t[:, tk, ts(tm, PT)],
                        lambda tk, c0: at_t[:, i, tk, c0 : c0 + RG],
                    )
                    if tm % 2 == 0:
                        nc.vector.tensor_copy(tt_t[:, i, tm, :], o1[:])
                    else:
                        nc.scalar.copy(tt_t[:, i, tm, :], o1[:])

            # Pass 2: Y = sum_i Tt_i^T @ B_i  -> [h, w] layout
            y_t = y_pool.tile([PT, NT, W], f32)
            for tm in range(NT):
                o2 = ps2.tile([PT, W], f32, name="o2", tag="o2")
                n_mm = r * sum(len(tks) for _, tks in REGIONS)
                cnt = 0
                for c0, tks in REGIONS:
                    for i in range(r):
                        for tk in tks:
                            nc.tensor.matmul(
                                o2[:, c0 : c0 + RG],
                                tt_t[:, i, tk, ts(tm, PT)],
                                b_t[:, i, tk, c0 : c0 + RG],
                                start=(cnt == 0),
                                stop=(cnt == n_mm - 1),
                            )
                            cnt += 1
                if tm % 2 == 0:
                    nc.vector.tensor_copy(y_t[:, tm, :], o2[:])
                else:
                    nc.scalar.copy(y_t[:, tm, :], o2[:])
                q = nc.scalar if tm % 2 == 0 else nc.sync
                q.dma_start(y_d.ap()[s, ts(tm, PT), :], y_t[:, tm, :])

    nc.compile()
    return nc


def _band(taps: np.ndarray, n: int) -> np.ndarray:
    """M[a, b] = taps[a - b + PAD] for |a - b| <= PAD, else 0."""
    M = np.zeros((n, n), np.float64)
    idx = np.arange(n)
    for d in range(-PAD, PAD + 1):
        b = idx[(idx + d >= 0) & (idx + d < n)]
        M[b + d, b] = taps[d + PAD]
    return M


def kernel(x: np.ndarray, kernel: np.ndarray) -> np.ndarray:
    x = np.asarray(x, dtype=np.float32)
    k2d = np.asarray(kernel, dtype=np.float32)
    n, c, h, w = x.shape
    assert (h, w) == (H, W) and k2d.shape == (KS, KS)

    # Separable decomposition (exact up to fp32 rounding for low-rank kernels).
    U, S, Vt = np.linalg.svd(k2d.astype(np.float64))
    r = max(1, int(np.sum(S > S[0] * 1e-7)))
    r = min(r, 8)

    at = np.empty((r, H, H), np.float32)
    bm = np.empty((r, W, W), np.float32)
    for i in range(r):
        kx = S[i] * U[:, i]  # taps along H
        ky = Vt[i]           # taps along W
        # Pass-1 rhs: AT[k, h] = kx[k - h + PAD]  (= band(kx))
        at[i] = _band(kx, H).astype(np.float32)
        # Pass-2 rhs: B[j, w2] = ky[j - w2 + PAD] (= band(ky))
        bm[i] = _band(ky, W).astype(np.float32)

    if r not in _kernel_cache:
        _kernel_cache[r] = _build_bass(r)
    nc = _kernel_cache[r]

    xr = x.reshape(n * c, H, W)
    per = xr.shape[0] // N_CORES
    in_maps = [
        {"x": np.ascontiguousarray(xr[ci * per : (ci + 1) * per]), "at": at, "b": bm}
        for ci in range(N_CORES)
    ]
    res = run_bass_kernel_spmd(nc, in_maps, core_ids=list(range(N_CORES)))
    global last_results
    last_results = res
    y = np.concatenate([res.results[ci]["y"] for ci in range(N_CORES)], axis=0)
    return y.reshape(n, c, h, w).astype(np.float32)


last_results = None

